# revision 4
# baseline (speedup 1.0000x reference)
"""Trainium2 Bass kernel for BasinCoupledQFIAttention.

kernel(**inputs) takes the FULL inputs (x:(4,512,128), basin:(128,), w_temp:(128,),
b_temp:(), residual_scale:()) and returns the full (4,512,128) output.

Sharding: 8 cores = 4 batches x 2 query-halves. Each core computes the full
Fisher-Rao attention for its 256 query rows against all 512 keys of its batch.
"""

import numpy as np
from contextlib import ExitStack

import concourse.bass as bass
import concourse.bacc as bacc
import concourse.tile as tile
from concourse import mybir
from concourse import bass_utils

B, T, D = 4, 512, 128
NCORES = 8
TQ = (B * T) // NCORES  # 256 query rows per core
NQB = TQ // 128         # query blocks of 128 per core
NKT = T // 128          # key tiles per batch
EPS = 1e-8
F32 = mybir.dt.float32
AF = mybir.ActivationFunctionType
ALU = mybir.AluOpType

GRP = 16  # queries per ACT sqrt instruction group (must divide 128)

_CACHE = {}


def _body(ctx: ExitStack, tc: tile.TileContext, aps: dict, red_dt):
    nc = tc.nc

    singles = ctx.enter_context(tc.tile_pool(name="singles", bufs=1))
    small = ctx.enter_context(tc.tile_pool(name="small", bufs=2))
    sbig_pool = ctx.enter_context(tc.tile_pool(name="sbig", bufs=2))
    st4 = ctx.enter_context(tc.tile_pool(name="st4", bufs=2))
    psum_inner = ctx.enter_context(tc.tile_pool(name="psin", bufs=2, space="PSUM"))
    psum_tp = ctx.enter_context(tc.tile_pool(name="pstp", bufs=2, space="PSUM"))
    psum_attn = ctx.enter_context(tc.tile_pool(name="psat", bufs=2, space="PSUM"))

    # ---- persistent SBUF tensors ----
    zsel = singles.tile([128, 255], F32, tag="zsel")
    ident = singles.tile([128, 128], F32, tag="ident")
    xkv = singles.tile([128, T], F32, tag="xkv")        # (k in tile, [kt, d])
    xq = singles.tile([128, TQ], F32, tag="xq")         # (q in blk, [qb, d])
    pnT = singles.tile([128, T], F32, tag="pnT")        # (d, keys)
    pnqT = singles.tile([128, TQ], F32, tag="pnqT")     # (d, queries)
    alpha_bc = singles.tile([128, 1], F32, tag="alpha_bc")
    rs_bc = singles.tile([128, 1], F32, tag="rs_bc")
    omr_bc = singles.tile([128, 1], F32, tag="omr_bc")
    zero_bc = singles.tile([128, 1], F32, tag="zero_bc")
    eps_bc = singles.tile([128, 1], F32, tag="eps_bc")
    one_bc = singles.tile([128, 1], F32, tag="one_bc")
    nc.gpsimd.memset(zero_bc[:], 0.0)
    nc.gpsimd.memset(eps_bc[:], EPS)
    nc.gpsimd.memset(one_bc[:], 1.0)

    nc.sync.dma_start(zsel[:], aps["zsel"])
    nc.sync.dma_start(ident[:], aps["ident"])
    nc.sync.dma_start(xkv[:].rearrange("p (kt d) -> p kt d", kt=NKT),
                      aps["xkv"].rearrange("(kt p) d -> p kt d", p=128))
    nc.sync.dma_start(xq[:].rearrange("p (qb d) -> p qb d", qb=NQB),
                      aps["xq"].rearrange("(qb p) d -> p qb d", p=128))

    # ---- stage 1: alpha = -2 / temperature ----
    basin = singles.tile([1, D], F32, tag="basin")
    wtemp = singles.tile([1, D], F32, tag="wtemp")
    btemp = singles.tile([1, 1], F32, tag="btemp")
    rs_s = singles.tile([1, 1], F32, tag="rs_s")
    nc.sync.dma_start(basin[:], aps["basin"])
    nc.sync.dma_start(wtemp[:], aps["w_temp"])
    nc.sync.dma_start(btemp[:], aps["b_temp"])
    nc.sync.dma_start(rs_s[:], aps["res_scale"])

    bw = small.tile([1, D], F32, tag="bw")
    nc.vector.tensor_tensor(out=bw[:], in0=basin[:], in1=wtemp[:], op=ALU.mult)
    dot = small.tile([1, 1], F32, tag="dot")
    nc.vector.tensor_reduce(out=dot[:], in_=bw[:], axis=mybir.AxisListType.X,
                            op=ALU.add)
    sg = small.tile([1, 1], F32, tag="sg")
    nc.scalar.activation(sg[:], dot[:], AF.Sigmoid, bias=btemp[:], scale=1.0)
    tau = small.tile([1, 1], F32, tag="tau")
    nc.vector.tensor_scalar(out=tau[:], in0=sg[:], scalar1=0.5, scalar2=1e-6,
                            op0=ALU.add, op1=ALU.max)
    rtau = small.tile([1, 1], F32, tag="rtau")
    nc.vector.reciprocal(rtau[:], tau[:])
    alpha = small.tile([1, 1], F32, tag="alpha")
    nc.vector.tensor_scalar(out=alpha[:], in0=rtau[:], scalar1=-2.0, scalar2=None,
                            op0=ALU.mult)
    nc.gpsimd.partition_broadcast(alpha_bc[:], alpha[:])
    nc.gpsimd.partition_broadcast(rs_bc[:], rs_s[:])
    # 1 - rs
    nc.vector.tensor_scalar(out=omr_bc[:], in0=rs_bc[:], scalar1=-1.0, scalar2=1.0,
                            op0=ALU.mult, op1=ALU.add)

    # ---- stage 2: simplex projection -> pn, pnT (both kv rows and q rows) ----
    def project(src, nblk, dstT):
        # src: (128, nblk*128) rows-on-partitions; dstT: (128, nblk*128) d-on-partitions
        ex = small.tile([128, nblk * 128], F32, tag=f"ex{nblk}")
        nc.scalar.activation(ex[:], src[:], AF.Exp, bias=zero_bc[:])
        sp = small.tile([128, nblk * 128], F32, tag=f"sp{nblk}")
        nc.scalar.activation(sp[:], ex[:], AF.Ln, bias=one_bc[:])
        sp3 = sp[:].rearrange("p (kt d) -> p kt d", kt=nblk)
        rsum = small.tile([128, nblk], F32, tag=f"rsum{nblk}")
        nc.vector.tensor_reduce(out=rsum[:], in_=sp3, axis=mybir.AxisListType.X,
                                op=ALU.add)
        rsum_e = small.tile([128, nblk], F32, tag=f"rsume{nblk}")
        nc.vector.tensor_scalar(out=rsum_e[:], in0=rsum[:], scalar1=EPS, scalar2=None,
                                op0=ALU.add)
        rcp = small.tile([128, nblk], F32, tag=f"rcp{nblk}")
        nc.vector.reciprocal(rcp[:], rsum_e[:])
        p = small.tile([128, nblk * 128], F32, tag=f"p{nblk}")
        for k in range(nblk):
            nc.vector.tensor_scalar(out=p[:, k * 128:(k + 1) * 128],
                                    in0=sp[:, k * 128:(k + 1) * 128],
                                    scalar1=rcp[:, k:k + 1], scalar2=EPS,
                                    op0=ALU.mult, op1=ALU.max)
        p3 = p[:].rearrange("p (kt d) -> p kt d", kt=nblk)
        rsum2 = small.tile([128, nblk], F32, tag=f"rsum2{nblk}")
        nc.vector.tensor_reduce(out=rsum2[:], in_=p3, axis=mybir.AxisListType.X,
                                op=ALU.add)
        rsum2e = small.tile([128, nblk], F32, tag=f"rsum2e{nblk}")
        nc.vector.tensor_scalar(out=rsum2e[:], in0=rsum2[:], scalar1=EPS,
                                scalar2=None, op0=ALU.add)
        rcp2 = small.tile([128, nblk], F32, tag=f"rcp2{nblk}")
        nc.vector.reciprocal(rcp2[:], rsum2e[:])
        pn = small.tile([128, nblk * 128], F32, tag=f"pn{nblk}")
        for k in range(nblk):
            nc.vector.tensor_scalar(out=pn[:, k * 128:(k + 1) * 128],
                                    in0=p[:, k * 128:(k + 1) * 128],
                                    scalar1=rcp2[:, k:k + 1], scalar2=None,
                                    op0=ALU.mult)
        # transpose each (rows,128d) block -> dstT (d, rows)
        for k in range(nblk):
            tp = psum_tp.tile([128, 128], F32, tag="tp")
            nc.tensor.transpose(tp[:], pn[:, k * 128:(k + 1) * 128], ident[:])
            nc.vector.tensor_copy(dstT[:, k * 128:(k + 1) * 128], tp[:])

    project(xkv, NKT, pnT)
    project(xq, NQB, pnqT)

    # ---- stage 3: inner(i,j) = sum_d sqrt(pn_i pn_j + eps)  -> PSUM (128q, 512k) ----
    inner_ps = []
    for qb in range(NQB):
        ips = psum_inner.tile([128, T], F32, tag="inner")
        inner_ps.append(ips)
        for g in range(128 // GRP):
            sb = sbig_pool.tile([128, GRP * T], F32, tag="sbig")
            for j in range(GRP):
                q = qb * 128 + g * GRP + j
                nc.vector.tensor_scalar(out=sb[:, j * T:(j + 1) * T], in0=pnT[:],
                                        scalar1=pnqT[:, q:q + 1], scalar2=None,
                                        op0=ALU.mult)
            nc.scalar.activation(sb[:], sb[:], AF.Sqrt, bias=eps_bc[:])
            for j in range(GRP):
                jj = g * GRP + j
                if red_dt == F32:
                    lhs = zsel[:, 127 - jj:255 - jj]
                    rhs = sb[:, j * T:(j + 1) * T]
                else:
                    lhs = zsel[:, 127 - jj:255 - jj].bitcast(red_dt)
                    rhs = sb[:, j * T:(j + 1) * T].bitcast(red_dt)
                nc.tensor.matmul(ips[:], lhs, rhs,
                                 start=(jj == 0), stop=(jj == 127),
                                 skip_group_check=True)

    # ---- stage 4: softmax over keys + attention + residual ----
    for qb in range(NQB):
        ips = inner_ps[qb]
        xc = st4.tile([128, T], F32, tag="xc")
        nc.vector.tensor_scalar(out=xc[:], in0=ips[:], scalar1=1.0 - 1e-6,
                                scalar2=-1.0 + 1e-6, op0=ALU.min, op1=ALU.max)
        x2 = st4.tile([128, T], F32, tag="x2")
        nc.scalar.activation(x2[:], xc[:], AF.Square, bias=zero_bc[:])
        tsq = st4.tile([128, T], F32, tag="tsq")
        nc.scalar.activation(tsq[:], x2[:], AF.Sqrt, bias=one_bc[:], scale=-1.0)
        rx = st4.tile([128, T], F32, tag="rx")
        nc.vector.reciprocal(rx[:], xc[:])
        ratio = st4.tile([128, T], F32, tag="ratio")
        nc.vector.tensor_tensor(out=ratio[:], in0=tsq[:], in1=rx[:], op=ALU.mult)
        th = st4.tile([128, T], F32, tag="th")
        nc.scalar.activation(th[:], ratio[:], AF.Arctan, bias=zero_bc[:])
        ee = st4.tile([128, T], F32, tag="ee")
        nc.scalar.activation(ee[:], th[:], AF.Exp, bias=zero_bc[:], scale=alpha_bc[:])
        den = st4.tile([128, 1], F32, tag="den")
        nc.vector.tensor_reduce(out=den[:], in_=ee[:], axis=mybir.AxisListType.X,
                                op=ALU.add)
        rden = st4.tile([128, 1], F32, tag="rden")
        nc.vector.reciprocal(rden[:], den[:])
        rsden = st4.tile([128, 1], F32, tag="rsden")
        nc.vector.tensor_tensor(out=rsden[:], in0=rden[:], in1=rs_bc[:], op=ALU.mult)

        eT = st4.tile([128, T], F32, tag="eT")
        for kt in range(NKT):
            tp = psum_tp.tile([128, 128], F32, tag="tp")
            nc.tensor.transpose(tp[:], ee[:, kt * 128:(kt + 1) * 128], ident[:])
            nc.vector.tensor_copy(eT[:, kt * 128:(kt + 1) * 128], tp[:])

        aps_t = psum_attn.tile([128, 128], F32, tag="attn")
        for kt in range(NKT):
            nc.tensor.matmul(aps_t[:], eT[:, kt * 128:(kt + 1) * 128],
                             xkv[:, kt * 128:(kt + 1) * 128],
                             start=(kt == 0), stop=(kt == NKT - 1),
                             skip_group_check=True)

        t1 = st4.tile([128, 128], F32, tag="t1")
        nc.vector.tensor_scalar(out=t1[:], in0=xq[:, qb * 128:(qb + 1) * 128],
                                scalar1=omr_bc[:], scalar2=None, op0=ALU.mult)
        ob = st4.tile([128, 128], F32, tag="ob")
        nc.vector.scalar_tensor_tensor(out=ob[:], in0=aps_t[:], scalar=rsden[:],
                                       in1=t1[:], op0=ALU.mult, op1=ALU.add)
        nc.sync.dma_start(
            aps["out"].rearrange("(qb p) d -> qb p d", p=128)[qb], ob[:])


def _build(red_dt=F32):
    nc = bacc.Bacc("TRN2", target_bir_lowering=False, debug=False,
                   num_devices=NCORES)
    aps = {
        "xq": nc.dram_tensor("xq", (TQ, D), F32, kind="ExternalInput").ap(),
        "xkv": nc.dram_tensor("xkv", (T, D), F32, kind="ExternalInput").ap(),
        "basin": nc.dram_tensor("basin", (1, D), F32, kind="ExternalInput").ap(),
        "w_temp": nc.dram_tensor("w_temp", (1, D), F32, kind="ExternalInput").ap(),
        "b_temp": nc.dram_tensor("b_temp", (1, 1), F32, kind="ExternalInput").ap(),
        "res_scale": nc.dram_tensor("res_scale", (1, 1), F32,
                                    kind="ExternalInput").ap(),
        "ident": nc.dram_tensor("ident", (D, D), F32, kind="ExternalInput").ap(),
        "zsel": nc.dram_tensor("zsel", (D, 255), F32, kind="ExternalInput").ap(),
        "out": nc.dram_tensor("out", (TQ, D), F32, kind="ExternalOutput").ap(),
    }
    with tile.TileContext(nc) as tc:
        with ExitStack() as ctx:
            _body(ctx, tc, aps, red_dt)
    nc.compile()
    return nc


def get_nc(red_dt=F32):
    key = str(red_dt)
    if key not in _CACHE:
        _CACHE[key] = _build(red_dt)
    return _CACHE[key]


def make_in_maps(x, basin, w_temp, b_temp, residual_scale):
    x = np.ascontiguousarray(np.asarray(x, dtype=np.float32))
    basin = np.asarray(basin, dtype=np.float32).reshape(1, D)
    w_temp = np.asarray(w_temp, dtype=np.float32).reshape(1, D)
    b_temp = np.asarray(b_temp, dtype=np.float32).reshape(1, 1)
    rs = np.asarray(residual_scale, dtype=np.float32).reshape(1, 1)
    ident = np.eye(D, dtype=np.float32)
    zsel = np.zeros((D, 255), dtype=np.float32)
    zsel[:, 127] = 1.0
    in_maps = []
    for c in range(NCORES):
        b, h = c // 2, c % 2
        in_maps.append({
            "xq": np.ascontiguousarray(x[b, h * TQ:(h + 1) * TQ, :]),
            "xkv": np.ascontiguousarray(x[b]),
            "basin": basin, "w_temp": w_temp, "b_temp": b_temp,
            "res_scale": rs, "ident": ident, "zsel": zsel,
        })
    return in_maps


def kernel(x, basin, w_temp, b_temp, residual_scale, **extra):
    nc = get_nc()
    in_maps = make_in_maps(x, basin, w_temp, b_temp, residual_scale)
    res = bass_utils.run_bass_kernel_spmd(nc, in_maps,
                                          core_ids=list(range(NCORES)))
    out = np.empty((B, T, D), dtype=np.float32)
    for c in range(NCORES):
        b, h = c // 2, c % 2
        out[b, h * TQ:(h + 1) * TQ, :] = res.results[c]["out"]
    return out


# revision 6
# speedup vs baseline: 1.0319x; 1.0319x over previous
"""Trainium2 Bass kernel for BasinCoupledQFIAttention.

kernel(**inputs) takes the FULL inputs (x:(4,512,128), basin:(128,), w_temp:(128,),
b_temp:(), residual_scale:()) and returns the full (4,512,128) output.

Sharding: 8 cores = 4 batches x 2 query-halves. Each core computes the full
Fisher-Rao attention for its 256 query rows against all 512 keys of its batch.
"""

import numpy as np
from contextlib import ExitStack

import concourse.bass as bass
import concourse.bacc as bacc
import concourse.tile as tile
from concourse import mybir
from concourse import bass_utils

B, T, D = 4, 512, 128
NCORES = 8
TQ = (B * T) // NCORES  # 256 query rows per core
NQB = TQ // 128         # query blocks of 128 per core
NKT = T // 128          # key tiles per batch
EPS = 1e-8
F32 = mybir.dt.float32
AF = mybir.ActivationFunctionType
ALU = mybir.AluOpType

GRP = 8  # queries per ACT sqrt instruction group (must divide 128)

_CACHE = {}


def _body(ctx: ExitStack, tc: tile.TileContext, aps: dict, red_dt):
    nc = tc.nc

    singles = ctx.enter_context(tc.tile_pool(name="singles", bufs=1))
    small = ctx.enter_context(tc.tile_pool(name="small", bufs=2))
    sbig_pool = ctx.enter_context(tc.tile_pool(name="sbig", bufs=2))
    st4 = ctx.enter_context(tc.tile_pool(name="st4", bufs=2))
    psum_inner = ctx.enter_context(tc.tile_pool(name="psin", bufs=2, space="PSUM"))
    psum_tp = ctx.enter_context(tc.tile_pool(name="pstp", bufs=2, space="PSUM"))
    psum_attn = ctx.enter_context(tc.tile_pool(name="psat", bufs=2, space="PSUM"))

    # ---- persistent SBUF tensors ----
    zsel = singles.tile([128, 255], F32, tag="zsel")
    ident = singles.tile([128, 128], F32, tag="ident")
    xkv = singles.tile([128, T], F32, tag="xkv")        # (k in tile, [kt, d])
    xq = singles.tile([128, TQ], F32, tag="xq")         # (q in blk, [qb, d])
    pnT = singles.tile([128, T], F32, tag="pnT")        # (d, keys)
    pnqT = singles.tile([128, TQ], F32, tag="pnqT")     # (d, queries)
    alpha_bc = singles.tile([128, 1], F32, tag="alpha_bc")
    rs_bc = singles.tile([128, 1], F32, tag="rs_bc")
    omr_bc = singles.tile([128, 1], F32, tag="omr_bc")
    zero_bc = singles.tile([128, 1], F32, tag="zero_bc")
    eps_bc = singles.tile([128, 1], F32, tag="eps_bc")
    one_bc = singles.tile([128, 1], F32, tag="one_bc")
    nc.gpsimd.memset(zero_bc[:], 0.0)
    nc.gpsimd.memset(eps_bc[:], EPS)
    nc.gpsimd.memset(one_bc[:], 1.0)

    nc.sync.dma_start(zsel[:], aps["zsel"])
    nc.sync.dma_start(ident[:], aps["ident"])
    nc.sync.dma_start(xkv[:].rearrange("p (kt d) -> p kt d", kt=NKT),
                      aps["xkv"].rearrange("(kt p) d -> p kt d", p=128))
    nc.sync.dma_start(xq[:].rearrange("p (qb d) -> p qb d", qb=NQB),
                      aps["xq"].rearrange("(qb p) d -> p qb d", p=128))

    # ---- stage 1: alpha = -2 / temperature ----
    basin = singles.tile([1, D], F32, tag="basin")
    wtemp = singles.tile([1, D], F32, tag="wtemp")
    btemp = singles.tile([1, 1], F32, tag="btemp")
    rs_s = singles.tile([1, 1], F32, tag="rs_s")
    nc.sync.dma_start(basin[:], aps["basin"])
    nc.sync.dma_start(wtemp[:], aps["w_temp"])
    nc.sync.dma_start(btemp[:], aps["b_temp"])
    nc.sync.dma_start(rs_s[:], aps["res_scale"])

    bw = small.tile([1, D], F32, tag="bw")
    nc.vector.tensor_tensor(out=bw[:], in0=basin[:], in1=wtemp[:], op=ALU.mult)
    dot = small.tile([1, 1], F32, tag="dot")
    nc.vector.tensor_reduce(out=dot[:], in_=bw[:], axis=mybir.AxisListType.X,
                            op=ALU.add)
    sg = small.tile([1, 1], F32, tag="sg")
    nc.scalar.activation(sg[:], dot[:], AF.Sigmoid, bias=btemp[:], scale=1.0)
    tau = small.tile([1, 1], F32, tag="tau")
    nc.vector.tensor_scalar(out=tau[:], in0=sg[:], scalar1=0.5, scalar2=1e-6,
                            op0=ALU.add, op1=ALU.max)
    rtau = small.tile([1, 1], F32, tag="rtau")
    nc.vector.reciprocal(rtau[:], tau[:])
    alpha = small.tile([1, 1], F32, tag="alpha")
    nc.vector.tensor_scalar(out=alpha[:], in0=rtau[:], scalar1=-2.0, scalar2=None,
                            op0=ALU.mult)
    nc.gpsimd.partition_broadcast(alpha_bc[:], alpha[:])
    nc.gpsimd.partition_broadcast(rs_bc[:], rs_s[:])
    # 1 - rs
    nc.vector.tensor_scalar(out=omr_bc[:], in0=rs_bc[:], scalar1=-1.0, scalar2=1.0,
                            op0=ALU.mult, op1=ALU.add)

    # ---- stage 2: simplex projection -> pn, pnT (both kv rows and q rows) ----
    def project(src, nblk, dstT):
        # src: (128, nblk*128) rows-on-partitions; dstT: (128, nblk*128) d-on-partitions
        ex = small.tile([128, nblk * 128], F32, tag=f"ex{nblk}")
        nc.scalar.activation(ex[:], src[:], AF.Exp, bias=zero_bc[:])
        sp = small.tile([128, nblk * 128], F32, tag=f"sp{nblk}")
        nc.scalar.activation(sp[:], ex[:], AF.Ln, bias=one_bc[:])
        sp3 = sp[:].rearrange("p (kt d) -> p kt d", kt=nblk)
        rsum = small.tile([128, nblk], F32, tag=f"rsum{nblk}")
        nc.vector.tensor_reduce(out=rsum[:], in_=sp3, axis=mybir.AxisListType.X,
                                op=ALU.add)
        rsum_e = small.tile([128, nblk], F32, tag=f"rsume{nblk}")
        nc.vector.tensor_scalar(out=rsum_e[:], in0=rsum[:], scalar1=EPS, scalar2=None,
                                op0=ALU.add)
        rcp = small.tile([128, nblk], F32, tag=f"rcp{nblk}")
        nc.vector.reciprocal(rcp[:], rsum_e[:])
        p = small.tile([128, nblk * 128], F32, tag=f"p{nblk}")
        for k in range(nblk):
            nc.vector.tensor_scalar(out=p[:, k * 128:(k + 1) * 128],
                                    in0=sp[:, k * 128:(k + 1) * 128],
                                    scalar1=rcp[:, k:k + 1], scalar2=EPS,
                                    op0=ALU.mult, op1=ALU.max)
        p3 = p[:].rearrange("p (kt d) -> p kt d", kt=nblk)
        rsum2 = small.tile([128, nblk], F32, tag=f"rsum2{nblk}")
        nc.vector.tensor_reduce(out=rsum2[:], in_=p3, axis=mybir.AxisListType.X,
                                op=ALU.add)
        rsum2e = small.tile([128, nblk], F32, tag=f"rsum2e{nblk}")
        nc.vector.tensor_scalar(out=rsum2e[:], in0=rsum2[:], scalar1=EPS,
                                scalar2=None, op0=ALU.add)
        rcp2 = small.tile([128, nblk], F32, tag=f"rcp2{nblk}")
        nc.vector.reciprocal(rcp2[:], rsum2e[:])
        pn = small.tile([128, nblk * 128], F32, tag=f"pn{nblk}")
        for k in range(nblk):
            nc.vector.tensor_scalar(out=pn[:, k * 128:(k + 1) * 128],
                                    in0=p[:, k * 128:(k + 1) * 128],
                                    scalar1=rcp2[:, k:k + 1], scalar2=None,
                                    op0=ALU.mult)
        # transpose each (rows,128d) block -> dstT (d, rows)
        for k in range(nblk):
            tp = psum_tp.tile([128, 128], F32, tag="tp")
            nc.tensor.transpose(tp[:], pn[:, k * 128:(k + 1) * 128], ident[:])
            nc.vector.tensor_copy(dstT[:, k * 128:(k + 1) * 128], tp[:])

    project(xkv, NKT, pnT)
    project(xq, NQB, pnqT)

    # ---- stage 3: inner(i,j) = sum_d sqrt(pn_i pn_j + eps)  -> PSUM (128q, 512k) ----
    if red_dt != F32:
        zsel_r = singles.tile([128, 255], red_dt, tag="zsel_r")
        nc.vector.tensor_copy(zsel_r[:], zsel[:])
    inner_ps = []
    for qb in range(NQB):
        ips = psum_inner.tile([128, T], F32, tag="inner")
        inner_ps.append(ips)
        for g in range(128 // GRP):
            pr = sbig_pool.tile([128, GRP * T], F32, tag="prod")
            for j in range(GRP):
                q = qb * 128 + g * GRP + j
                nc.vector.tensor_scalar(out=pr[:, j * T:(j + 1) * T], in0=pnT[:],
                                        scalar1=pnqT[:, q:q + 1], scalar2=None,
                                        op0=ALU.mult)
            sb = sbig_pool.tile([128, GRP * T], red_dt, tag="sbig")
            nc.scalar.activation(sb[:], pr[:], AF.Sqrt, bias=eps_bc[:])
            for j in range(GRP):
                jj = g * GRP + j
                lhs = zsel[:, 127 - jj:255 - jj] if red_dt == F32 \
                    else zsel_r[:, 127 - jj:255 - jj]
                nc.tensor.matmul(ips[:], lhs, sb[:, j * T:(j + 1) * T],
                                 start=(jj == 0), stop=(jj == 127),
                                 skip_group_check=True)

    # ---- stage 4: softmax over keys + attention + residual ----
    for qb in range(NQB):
        ips = inner_ps[qb]
        xc = st4.tile([128, T], F32, tag="xc")
        nc.vector.tensor_scalar(out=xc[:], in0=ips[:], scalar1=1.0 - 1e-6,
                                scalar2=-1.0 + 1e-6, op0=ALU.min, op1=ALU.max)
        x2 = st4.tile([128, T], F32, tag="x2")
        nc.scalar.activation(x2[:], xc[:], AF.Square, bias=zero_bc[:])
        tsq = st4.tile([128, T], F32, tag="tsq")
        nc.scalar.activation(tsq[:], x2[:], AF.Sqrt, bias=one_bc[:], scale=-1.0)
        rx = st4.tile([128, T], F32, tag="rx")
        nc.vector.reciprocal(rx[:], xc[:])
        ratio = st4.tile([128, T], F32, tag="ratio")
        nc.vector.tensor_tensor(out=ratio[:], in0=tsq[:], in1=rx[:], op=ALU.mult)
        th = st4.tile([128, T], F32, tag="th")
        nc.scalar.activation(th[:], ratio[:], AF.Arctan, bias=zero_bc[:])
        ee = st4.tile([128, T], F32, tag="ee")
        nc.scalar.activation(ee[:], th[:], AF.Exp, bias=zero_bc[:], scale=alpha_bc[:])
        den = st4.tile([128, 1], F32, tag="den")
        nc.vector.tensor_reduce(out=den[:], in_=ee[:], axis=mybir.AxisListType.X,
                                op=ALU.add)
        rden = st4.tile([128, 1], F32, tag="rden")
        nc.vector.reciprocal(rden[:], den[:])
        rsden = st4.tile([128, 1], F32, tag="rsden")
        nc.vector.tensor_tensor(out=rsden[:], in0=rden[:], in1=rs_bc[:], op=ALU.mult)

        eT = st4.tile([128, T], F32, tag="eT")
        for kt in range(NKT):
            tp = psum_tp.tile([128, 128], F32, tag="tp")
            nc.tensor.transpose(tp[:], ee[:, kt * 128:(kt + 1) * 128], ident[:])
            nc.vector.tensor_copy(eT[:, kt * 128:(kt + 1) * 128], tp[:])

        aps_t = psum_attn.tile([128, 128], F32, tag="attn")
        for kt in range(NKT):
            nc.tensor.matmul(aps_t[:], eT[:, kt * 128:(kt + 1) * 128],
                             xkv[:, kt * 128:(kt + 1) * 128],
                             start=(kt == 0), stop=(kt == NKT - 1),
                             skip_group_check=True)

        t1 = st4.tile([128, 128], F32, tag="t1")
        nc.vector.tensor_scalar(out=t1[:], in0=xq[:, qb * 128:(qb + 1) * 128],
                                scalar1=omr_bc[:], scalar2=None, op0=ALU.mult)
        ob = st4.tile([128, 128], F32, tag="ob")
        nc.vector.scalar_tensor_tensor(out=ob[:], in0=aps_t[:], scalar=rsden[:],
                                       in1=t1[:], op0=ALU.mult, op1=ALU.add)
        nc.sync.dma_start(
            aps["out"].rearrange("(qb p) d -> qb p d", p=128)[qb], ob[:])


def _build(red_dt=F32):
    nc = bacc.Bacc("TRN2", target_bir_lowering=False, debug=False,
                   num_devices=NCORES)
    aps = {
        "xq": nc.dram_tensor("xq", (TQ, D), F32, kind="ExternalInput").ap(),
        "xkv": nc.dram_tensor("xkv", (T, D), F32, kind="ExternalInput").ap(),
        "basin": nc.dram_tensor("basin", (1, D), F32, kind="ExternalInput").ap(),
        "w_temp": nc.dram_tensor("w_temp", (1, D), F32, kind="ExternalInput").ap(),
        "b_temp": nc.dram_tensor("b_temp", (1, 1), F32, kind="ExternalInput").ap(),
        "res_scale": nc.dram_tensor("res_scale", (1, 1), F32,
                                    kind="ExternalInput").ap(),
        "ident": nc.dram_tensor("ident", (D, D), F32, kind="ExternalInput").ap(),
        "zsel": nc.dram_tensor("zsel", (D, 255), F32, kind="ExternalInput").ap(),
        "out": nc.dram_tensor("out", (TQ, D), F32, kind="ExternalOutput").ap(),
    }
    with tile.TileContext(nc) as tc:
        with ExitStack() as ctx:
            _body(ctx, tc, aps, red_dt)
    nc.compile()
    return nc


def get_nc(red_dt=F32):
    key = str(red_dt)
    if key not in _CACHE:
        _CACHE[key] = _build(red_dt)
    return _CACHE[key]


def make_in_maps(x, basin, w_temp, b_temp, residual_scale):
    x = np.ascontiguousarray(np.asarray(x, dtype=np.float32))
    basin = np.asarray(basin, dtype=np.float32).reshape(1, D)
    w_temp = np.asarray(w_temp, dtype=np.float32).reshape(1, D)
    b_temp = np.asarray(b_temp, dtype=np.float32).reshape(1, 1)
    rs = np.asarray(residual_scale, dtype=np.float32).reshape(1, 1)
    ident = np.eye(D, dtype=np.float32)
    zsel = np.zeros((D, 255), dtype=np.float32)
    zsel[:, 127] = 1.0
    in_maps = []
    for c in range(NCORES):
        b, h = c // 2, c % 2
        in_maps.append({
            "xq": np.ascontiguousarray(x[b, h * TQ:(h + 1) * TQ, :]),
            "xkv": np.ascontiguousarray(x[b]),
            "basin": basin, "w_temp": w_temp, "b_temp": b_temp,
            "res_scale": rs, "ident": ident, "zsel": zsel,
        })
    return in_maps


import os as _os
RED_DT_DEFAULT = (mybir.dt.float32r if _os.environ.get("KERNEL_F32R", "0") == "1"
                  else F32)


def kernel(x, basin, w_temp, b_temp, residual_scale, **extra):
    nc = get_nc(RED_DT_DEFAULT)
    in_maps = make_in_maps(x, basin, w_temp, b_temp, residual_scale)
    res = bass_utils.run_bass_kernel_spmd(nc, in_maps,
                                          core_ids=list(range(NCORES)))
    out = np.empty((B, T, D), dtype=np.float32)
    for c in range(NCORES):
        b, h = c // 2, c % 2
        out[b, h * TQ:(h + 1) * TQ, :] = res.results[c]["out"]
    return out
